# revision 4
# baseline (speedup 1.0000x reference)
"""AttentionHead with positional-bias matrices, 8-core Trainium2 Bass kernel.

Math (per reference):
  q = query @ Wq.T + bq           [B,S,D]
  k = key   @ Wk.T + bk           [B,S,D]
  v = value @ Wv.T + bv           [B,S,D]
  scores[b,s,t] = (q[b,s]·k[b,t] + q[b,s]·k_bias[s,t]) / sqrt(D) + maskadd[b,t]
  w = softmax_t(scores)
  out[b,s,:] = w[b,s,:] @ v[b] + sum_t w[b,s,t]*v_bias[s,t,:]

Sharding: sequence-parallel over the query-position axis s. Core c owns
s in [c*128, (c+1)*128) for ALL batches. The [S,S,D] bias matrices are
read exactly once globally (each core reads only its s-slice). k/v
projections are computed redundantly on every core (no collectives).

Host-side prep: downcast to bf16 (halves HBM traffic; matmuls accumulate
in f32 PSUM) and pre-transpose activations to [H, B*S] so the contraction
dim h lands on SBUF partitions without any on-device transposes.
1/sqrt(D) is folded into Wq/bq on the host.
"""

import os
import math
import numpy as np
import ml_dtypes

import concourse.bass as bass
import concourse.mybir as mybir
import concourse.tile as tile
from concourse import bacc
from concourse.masks import make_identity
from concourse.bass_utils import run_bass_kernel_spmd

B, S, H, D = 16, 1024, 1024, 128
NCORES = 8
SSL = S // NCORES          # query positions per core
BS = B * S                 # 16384
BSL = B * SSL              # 2048
HO = H // 128              # 8 h-chunks
TC = S // 128              # 8 t-chunks
PCHUNK = 512               # projection (b,t) chunk
NPCH = BS // PCHUNK        # 32
NQCH = BSL // PCHUNK       # 4

BF16 = mybir.dt.bfloat16
F32 = mybir.dt.float32

_cache = {}


def _build_nc():
    nc = bacc.Bacc()

    # ---- per-core inputs (bf16 unless noted) ----
    queryT = nc.dram_tensor("queryT", [H, BSL], BF16, kind="ExternalInput")
    keyT = nc.dram_tensor("keyT", [H, BS], BF16, kind="ExternalInput")
    valueT = nc.dram_tensor("valueT", [H, BS], BF16, kind="ExternalInput")
    WqT = nc.dram_tensor("WqT", [H, D], BF16, kind="ExternalInput")
    WkT = nc.dram_tensor("WkT", [H, D], BF16, kind="ExternalInput")
    WvT = nc.dram_tensor("WvT", [H, D], BF16, kind="ExternalInput")
    bq = nc.dram_tensor("bq", [D], F32, kind="ExternalInput")
    bk = nc.dram_tensor("bk", [D], F32, kind="ExternalInput")
    bv = nc.dram_tensor("bv", [D], F32, kind="ExternalInput")
    kbT = nc.dram_tensor("kbT", [SSL, D, S], BF16, kind="ExternalInput")
    vb = nc.dram_tensor("vb", [SSL, S, D], BF16, kind="ExternalInput")
    maskadd = nc.dram_tensor("maskadd", [B, S], F32, kind="ExternalInput")
    out_h = nc.dram_tensor("out", [B, SSL, D], F32, kind="ExternalOutput")

    qT_r = queryT.rearrange("(ho p) n -> p ho n", p=128)
    keyT_r = keyT.rearrange("(ho p) n -> p ho n", p=128)
    valueT_r = valueT.rearrange("(ho p) n -> p ho n", p=128)

    with tile.TileContext(nc) as tc:
        with (
            tc.tile_pool(name="const", bufs=1) as constp,
            tc.tile_pool(name="big", bufs=1) as bigp,
            tc.tile_pool(name="stream", bufs=2) as streamp,
            tc.tile_pool(name="evac", bufs=3) as evacp,
        ):
            # ---- resident SBUF tensors ----
            kT_sb = bigp.tile([128, B, S], BF16)          # [d, b, t]    32KB/part
            v_sb = bigp.tile([128, B, TC, 128], BF16)     # [tp, b, tc, d] 32KB
            qT_sb = bigp.tile([128, B, SSL], BF16)        # [d, b, s]    4KB
            a2buf = bigp.tile([128, B, S], BF16)          # [s, b, t]    32KB
            eT_sb = bigp.tile([128, TC, B, SSL], BF16)    # [tp, tc, b, s] 32KB
            v2buf = bigp.tile([128, B, D], BF16)          # [s, b, d]    4KB
            outbuf = bigp.tile([128, B, D], F32)          # [s, b, d]    8KB
            rowsum = bigp.tile([128, B], F32)
            recip = bigp.tile([128, B], F32)

            wq_sb = constp.tile([128, HO, D], BF16)
            wk_sb = constp.tile([128, HO, D], BF16)
            wv_sb = constp.tile([128, HO, D], BF16)
            bq_sb = constp.tile([128, 1], F32)
            bk_sb = constp.tile([128, 1], F32)
            bv_sb = constp.tile([128, 1], F32)
            mask_sb = constp.tile([B, S], F32)
            ident = constp.tile([128, 128], BF16)

            nc.sync.dma_start(wq_sb[:], WqT.rearrange("(ho p) d -> p ho d", p=128))
            nc.sync.dma_start(wk_sb[:], WkT.rearrange("(ho p) d -> p ho d", p=128))
            nc.sync.dma_start(wv_sb[:], WvT.rearrange("(ho p) d -> p ho d", p=128))
            nc.sync.dma_start(bq_sb[:], bq.rearrange("(o p) -> p o", p=128))
            nc.sync.dma_start(bk_sb[:], bk.rearrange("(o p) -> p o", p=128))
            nc.sync.dma_start(bv_sb[:], bv.rearrange("(o p) -> p o", p=128))
            nc.sync.dma_start(mask_sb[:], maskadd[:, :])
            make_identity(nc, ident[:])

            # ================= P0/P2: projections =================
            # qT/kT: out[d, (b,t)-chunk] = sum_h W[d,h] * xT[h, chunk]
            with (
                tc.tile_pool(name="mmps", bufs=2, space="PSUM") as mmps,
                tc.tile_pool(name="tps", bufs=2, space="PSUM") as tps,
            ):
                # q projection (scale folded into WqT/bq on host)
                for c in range(NQCH):
                    xt = streamp.tile([128, HO, PCHUNK], BF16, tag="xchunk")
                    nc.sync.dma_start(xt[:], qT_r[:, :, c * PCHUNK:(c + 1) * PCHUNK])
                    ps = mmps.tile([128, PCHUNK], F32, tag="mm")
                    for ho in range(HO):
                        nc.tensor.matmul(ps[:], lhsT=wq_sb[:, ho, :], rhs=xt[:, ho, :],
                                         start=(ho == 0), stop=(ho == HO - 1))
                    nc.scalar.activation(
                        qT_sb.rearrange("p b s -> p (b s)")[:, c * PCHUNK:(c + 1) * PCHUNK],
                        ps[:], mybir.ActivationFunctionType.Identity,
                        bias=bq_sb[:], scale=1.0)

                # k projection -> kT_sb [d, (b,t)]
                for c in range(NPCH):
                    xt = streamp.tile([128, HO, PCHUNK], BF16, tag="xchunk")
                    nc.sync.dma_start(xt[:], keyT_r[:, :, c * PCHUNK:(c + 1) * PCHUNK])
                    ps = mmps.tile([128, PCHUNK], F32, tag="mm")
                    for ho in range(HO):
                        nc.tensor.matmul(ps[:], lhsT=wk_sb[:, ho, :], rhs=xt[:, ho, :],
                                         start=(ho == 0), stop=(ho == HO - 1))
                    nc.scalar.activation(
                        kT_sb.rearrange("p b t -> p (b t)")[:, c * PCHUNK:(c + 1) * PCHUNK],
                        ps[:], mybir.ActivationFunctionType.Identity,
                        bias=bk_sb[:], scale=1.0)

                # v projection -> vT chunk, then PE-transpose into v_sb [tp, b, tc, d]
                for c in range(NPCH):
                    xt = streamp.tile([128, HO, PCHUNK], BF16, tag="xchunk")
                    nc.sync.dma_start(xt[:], valueT_r[:, :, c * PCHUNK:(c + 1) * PCHUNK])
                    ps = mmps.tile([128, PCHUNK], F32, tag="mm")
                    for ho in range(HO):
                        nc.tensor.matmul(ps[:], lhsT=wv_sb[:, ho, :], rhs=xt[:, ho, :],
                                         start=(ho == 0), stop=(ho == HO - 1))
                    vt_tmp = evacp.tile([128, PCHUNK], BF16, tag="vt_tmp")
                    nc.scalar.activation(vt_tmp[:], ps[:],
                                         mybir.ActivationFunctionType.Identity,
                                         bias=bv_sb[:], scale=1.0)
                    b_idx = c // 2
                    for i in range(PCHUNK // 128):
                        tcg = (c % 2) * 4 + i
                        tp_ps = tps.tile([128, 128], BF16, tag="tp")
                        nc.tensor.transpose(tp_ps[:], vt_tmp[:, i * 128:(i + 1) * 128],
                                            ident[:])
                        nc.vector.tensor_copy(out=v_sb[:, b_idx, tcg, :], in_=tp_ps[:])

            # ================= P1: attn_2 (bias scores), per s =================
            # a2[b, t] = sum_d q[b, s, d] * k_bias[s, t, d]
            with tc.tile_pool(name="a2ps", bufs=3, space="PSUM") as a2ps:
                for s in range(SSL):
                    kbt = streamp.tile([128, S], BF16, tag="kbt", bufs=3)
                    nc.sync.dma_start(kbt[:], kbT[s, :, :])
                    ps = a2ps.tile([B, S], F32, tag="a2")
                    for h in range(2):
                        nc.tensor.matmul(ps[:, h * 512:(h + 1) * 512],
                                         lhsT=qT_sb[:, :, s], rhs=kbt[:, h * 512:(h + 1) * 512],
                                         start=True, stop=True)
                    # evac + mask add (maskadd[b,t], broadcast over s) -> bf16
                    ev = evacp.tile([B, S], BF16, tag="a2evac")
                    nc.vector.tensor_add(out=ev[:], in0=ps[:], in1=mask_sb[:])
                    # row-shuffle: [b, t] rows -> partition s of a2buf
                    nc.sync.dma_start(a2buf[s:s + 1, :, :], ev[:])

            # ================= P3a: scores + softmax + eT, per b =================
            with (
                tc.tile_pool(name="scps", bufs=2, space="PSUM") as scps,
                tc.tile_pool(name="tps2", bufs=2, space="PSUM") as tps2,
            ):
                for b in range(B):
                    ps = scps.tile([128, S], F32, tag="sc")
                    for h in range(2):
                        sl = slice(h * 512, (h + 1) * 512)
                        nc.tensor.matmul(ps[:, sl], lhsT=qT_sb[:, b, :],
                                         rhs=kT_sb[:, b, sl], start=True, stop=False)
                        nc.tensor.matmul(ps[:, sl], lhsT=ident[:],
                                         rhs=a2buf[:, b, sl], start=False, stop=True)
                    e_sb = evacp.tile([128, S], BF16, tag="e", bufs=2)
                    nc.scalar.activation(e_sb[:], ps[:],
                                         mybir.ActivationFunctionType.Exp,
                                         bias=0.0, scale=1.0,
                                         accum_out=rowsum[:, b:b + 1])
                    for t in range(TC):
                        tp_ps = tps2.tile([128, 128], BF16, tag="tp2")
                        nc.tensor.transpose(tp_ps[:], e_sb[:, t * 128:(t + 1) * 128],
                                            ident[:])
                        nc.vector.tensor_copy(out=eT_sb[:, t, b, :], in_=tp_ps[:])
                nc.vector.reciprocal(recip[:], rowsum[:])

            # ================= P4: values_2 (bias values), per s =================
            # v2[b, d] = sum_t e[b, s, t] * v_bias[s, t, d]
            with tc.tile_pool(name="v2ps", bufs=3, space="PSUM") as v2ps:
                for s in range(SSL):
                    vbt = streamp.tile([128, TC, D], BF16, tag="vbt", bufs=3)
                    nc.sync.dma_start(vbt[:], vb[s].rearrange("(tc tp) d -> tp tc d", tp=128))
                    ps = v2ps.tile([B, D], F32, tag="v2")
                    for t in range(TC):
                        nc.tensor.matmul(ps[:], lhsT=eT_sb[:, t, :, s], rhs=vbt[:, t, :],
                                         start=(t == 0), stop=(t == TC - 1))
                    ev = evacp.tile([B, D], BF16, tag="v2evac")
                    nc.vector.tensor_copy(out=ev[:], in_=ps[:])
                    nc.sync.dma_start(v2buf[s:s + 1, :, :], ev[:])

            # ================= P3b: values_1 + combine + out =================
            with tc.tile_pool(name="ops", bufs=2, space="PSUM") as ops:
                for b in range(B):
                    ps = ops.tile([128, D], F32, tag="o")
                    for t in range(TC):
                        nc.tensor.matmul(ps[:], lhsT=eT_sb[:, t, b, :],
                                         rhs=v_sb[:, b, t, :],
                                         start=(t == 0), stop=False)
                    nc.tensor.matmul(ps[:], lhsT=ident[:], rhs=v2buf[:, b, :],
                                     start=False, stop=True)
                    nc.scalar.activation(outbuf[:, b, :], ps[:],
                                         mybir.ActivationFunctionType.Copy,
                                         bias=0.0, scale=recip[:, b:b + 1])
                nc.sync.dma_start(out_h.rearrange("b s d -> s b d"), outbuf[:])

    nc.finalize()
    return nc


def _prep_inputs(query, key, value, mask, Wq, bq, Wk, bk, Wv, bv, k_bias, v_bias):
    scale = 1.0 / math.sqrt(D)
    bf = ml_dtypes.bfloat16
    keyT = np.ascontiguousarray(key.transpose(2, 0, 1).reshape(H, BS)).astype(bf)
    valueT = np.ascontiguousarray(value.transpose(2, 0, 1).reshape(H, BS)).astype(bf)
    WqTs = np.ascontiguousarray((Wq.T * scale)).astype(bf)
    WkT = np.ascontiguousarray(Wk.T).astype(bf)
    WvT = np.ascontiguousarray(Wv.T).astype(bf)
    bqs = (bq * scale).astype(np.float32)
    maskadd = np.where(mask == 0, np.float32(-1e9), np.float32(0.0)).astype(np.float32)

    in_maps = []
    for c in range(NCORES):
        ssl = slice(c * SSL, (c + 1) * SSL)
        queryT = np.ascontiguousarray(
            query[:, ssl, :].transpose(2, 0, 1).reshape(H, BSL)).astype(bf)
        kbT = np.ascontiguousarray(k_bias[ssl].transpose(0, 2, 1)).astype(bf)
        vbc = np.ascontiguousarray(v_bias[ssl]).astype(bf)
        in_maps.append(dict(
            queryT=queryT, keyT=keyT, valueT=valueT,
            WqT=WqTs, WkT=WkT, WvT=WvT,
            bq=bqs, bk=bk.astype(np.float32), bv=bv.astype(np.float32),
            kbT=kbT, vb=vbc, maskadd=maskadd,
        ))
    return in_maps


def kernel(**inputs):
    if "nc" not in _cache:
        _cache["nc"] = _build_nc()
    nc = _cache["nc"]
    in_maps = _prep_inputs(**{k: np.asarray(v) for k, v in inputs.items()})
    trace = bool(int(os.environ.get("ATTN_TRACE", "0")))
    res = run_bass_kernel_spmd(nc, in_maps, core_ids=list(range(NCORES)),
                               trace=trace)
    if trace and res.exec_time_ns is not None:
        print(f"HW exec time: {res.exec_time_ns} ns")
        _cache["exec_time_ns"] = res.exec_time_ns
        _cache["trace"] = res.instructions_and_trace
    out = np.concatenate([r["out"] for r in res.results], axis=1)
    return out


# revision 11
# speedup vs baseline: 2.2633x; 2.2633x over previous
"""AttentionHead with positional-bias matrices, 8-core Trainium2 Bass kernel.

Math (per reference):
  q = query @ Wq.T + bq           [B,S,D]
  k = key   @ Wk.T + bk           [B,S,D]
  v = value @ Wv.T + bv           [B,S,D]
  scores[b,s,t] = (q[b,s]·k[b,t] + q[b,s]·k_bias[s,t]) / sqrt(D) + maskadd[b,t]
  w = softmax_t(scores)
  out[b,s,:] = w[b,s,:] @ v[b] + sum_t w[b,s,t]*v_bias[s,t,:]

Sharding: sequence-parallel over the query-position axis s. Core c owns
s in [c*128, (c+1)*128) for ALL batches. The [S,S,D] bias matrices are
read exactly once globally (each core reads only its s-slice). k/v
projections are computed redundantly on every core (no collectives).

Host-side prep: downcast to bf16 (halves HBM traffic; matmuls accumulate
in f32 PSUM) and pre-transpose activations to [H, B*S] so the contraction
dim h lands on SBUF partitions without any on-device transposes.
1/sqrt(D) is folded into Wq/bq on the host.
"""

import os
import math
import numpy as np
import ml_dtypes

import concourse.bass as bass
import concourse.mybir as mybir
import concourse.tile as tile
from concourse import bacc
from concourse.masks import make_identity
from concourse.bass_utils import run_bass_kernel_spmd

B, S, H, D = 16, 1024, 1024, 128
NCORES = 8
SSL = S // NCORES          # query positions per core
BS = B * S                 # 16384
BSL = B * SSL              # 2048
HO = H // 128              # 8 h-chunks
TC = S // 128              # 8 t-chunks
PCHUNK = 512               # projection (b,t) chunk
NPCH = BS // PCHUNK        # 32
NQCH = BSL // PCHUNK       # 4

BF16 = mybir.dt.bfloat16
F32 = mybir.dt.float32

_cache = {}


def _build_nc(mask_allones=True):
    nc = bacc.Bacc()

    # ---- per-core inputs (bf16 unless noted) ----
    queryT = nc.dram_tensor("queryT", [H, BSL], BF16, kind="ExternalInput")
    keyT = nc.dram_tensor("keyT", [H, BS], BF16, kind="ExternalInput")
    valueT = nc.dram_tensor("valueT", [H, BS], BF16, kind="ExternalInput")
    WqT = nc.dram_tensor("WqT", [H, D], BF16, kind="ExternalInput")
    WkT = nc.dram_tensor("WkT", [H, D], BF16, kind="ExternalInput")
    WvT = nc.dram_tensor("WvT", [H, D], BF16, kind="ExternalInput")
    bq = nc.dram_tensor("bq", [D], F32, kind="ExternalInput")
    bk = nc.dram_tensor("bk", [D], F32, kind="ExternalInput")
    bv = nc.dram_tensor("bv", [D], F32, kind="ExternalInput")
    kbT = nc.dram_tensor("kbT", [SSL, D, S], BF16, kind="ExternalInput")
    # vb host-permuted: [g, tp, sl, tc, d] with s = 2g+sl, t = tc*128+tp
    vb = nc.dram_tensor("vb", [SSL // 2, 128, 2, TC, D], BF16, kind="ExternalInput")
    maskadd = nc.dram_tensor("maskadd", [B, S], F32, kind="ExternalInput")
    out_h = nc.dram_tensor("out", [B, SSL, D], F32, kind="ExternalOutput")

    qT_r = queryT.rearrange("(ho p) n -> p ho n", p=128)
    keyT_r = keyT.rearrange("(ho p) n -> p ho n", p=128)
    valueT_r = valueT.rearrange("(ho p) n -> p ho n", p=128)

    with tile.TileContext(nc) as tc:
        with (
            tc.tile_pool(name="const", bufs=1) as constp,
            tc.tile_pool(name="big", bufs=1) as bigp,
            tc.tile_pool(name="stream", bufs=2) as streamp,
            tc.tile_pool(name="evac", bufs=3) as evacp,
        ):
            # ---- resident SBUF tensors ----
            kT_sb = bigp.tile([128, B, S], BF16)          # [d, b, t]    32KB/part
            v_sb = bigp.tile([128, B, TC, 128], BF16)     # [tp, b, tc, d] 32KB
            qT_sb = bigp.tile([128, B, SSL], BF16)        # [d, b, s]    4KB
            a2buf = bigp.tile([128, B, S], BF16)          # [s, b, t]    32KB
            eT_sb = bigp.tile([128, TC, B, SSL], BF16)    # [tp, tc, b, s] 32KB
            v2buf = bigp.tile([128, B, D], BF16)          # [s, b, d]    4KB
            outbuf = bigp.tile([128, B, D], F32)          # [s, b, d]    8KB
            rowsum = bigp.tile([128, B], F32)
            recip = bigp.tile([128, B], F32)

            wq_sb = constp.tile([128, HO, D], BF16)
            wk_sb = constp.tile([128, HO, D], BF16)
            wv_sb = constp.tile([128, HO, D], BF16)
            bq_sb = constp.tile([128, 1], F32)
            bk_sb = constp.tile([128, 1], F32)
            bv_sb = constp.tile([128, 1], F32)
            mask_sb = constp.tile([B, S], F32)
            ident = constp.tile([128, 128], BF16)

            nc.sync.dma_start(wq_sb[:], WqT.rearrange("(ho p) d -> p ho d", p=128))
            nc.sync.dma_start(wk_sb[:], WkT.rearrange("(ho p) d -> p ho d", p=128))
            nc.sync.dma_start(wv_sb[:], WvT.rearrange("(ho p) d -> p ho d", p=128))
            nc.sync.dma_start(bq_sb[:], bq.rearrange("(o p) -> p o", p=128))
            nc.sync.dma_start(bk_sb[:], bk.rearrange("(o p) -> p o", p=128))
            nc.sync.dma_start(bv_sb[:], bv.rearrange("(o p) -> p o", p=128))
            nc.sync.dma_start(mask_sb[:], maskadd[:, :])
            make_identity(nc, ident[:])

            # ========== P0/P1/P2: projections interleaved with attn_2 ==========
            # projection chunk: out[d, (b,t)-chunk] = sum_h W[d,h] * xT[h, chunk]
            def proj_chunk(kind, c, mmps, tps):
                src = {"q": qT_r, "k": keyT_r, "v": valueT_r}[kind]
                w = {"q": wq_sb, "k": wk_sb, "v": wv_sb}[kind]
                bias = {"q": bq_sb, "k": bk_sb, "v": bv_sb}[kind]
                xt = streamp.tile([128, HO, PCHUNK], BF16, tag="xchunk")
                nc.sync.dma_start(xt[:], src[:, :, c * PCHUNK:(c + 1) * PCHUNK])
                ps = mmps.tile([128, PCHUNK], F32, tag="mm")
                for ho in range(HO):
                    nc.tensor.matmul(ps[:], lhsT=w[:, ho, :], rhs=xt[:, ho, :],
                                     start=(ho == 0), stop=(ho == HO - 1))
                if kind == "q":
                    dst = qT_sb.rearrange("p b s -> p (b s)")
                    nc.scalar.activation(dst[:, c * PCHUNK:(c + 1) * PCHUNK], ps[:],
                                         mybir.ActivationFunctionType.Identity,
                                         bias=bias[:], scale=1.0)
                elif kind == "k":
                    dst = kT_sb.rearrange("p b t -> p (b t)")
                    nc.scalar.activation(dst[:, c * PCHUNK:(c + 1) * PCHUNK], ps[:],
                                         mybir.ActivationFunctionType.Identity,
                                         bias=bias[:], scale=1.0)
                else:
                    vt_tmp = evacp.tile([128, PCHUNK], BF16, tag="vt_tmp")
                    nc.scalar.activation(vt_tmp[:], ps[:],
                                         mybir.ActivationFunctionType.Identity,
                                         bias=bias[:], scale=1.0)
                    b_idx = c // 2
                    for i in range(PCHUNK // 128):
                        tcg = (c % 2) * 4 + i
                        tp_ps = tps.tile([128, 128], BF16, tag="tp")
                        nc.tensor.transpose(tp_ps[:], vt_tmp[:, i * 128:(i + 1) * 128],
                                            ident[:])
                        nc.vector.tensor_copy(out=v_sb[:, b_idx, tcg, :], in_=tp_ps[:])

            # attn_2 per s: a2[b, t] = sum_d q[b, s, d] * k_bias[s, t, d]
            # kbT streamed in 2-s groups (1MB DMAs); shuffle DMAs on SWDGE
            def attn2_group(g, a2ps):
                kbt = streamp.tile([128, 2, S], BF16, tag="kbt", bufs=2)
                nc.sync.dma_start(kbt[:], kbT[2 * g:2 * g + 2].rearrange("s d t -> d s t"))
                for si in range(2):
                    s = 2 * g + si
                    ps = a2ps.tile([B, S], F32, tag="a2")
                    for h in range(2):
                        nc.tensor.matmul(ps[:, h * 512:(h + 1) * 512],
                                         lhsT=qT_sb[:, :, s],
                                         rhs=kbt[:, si, h * 512:(h + 1) * 512],
                                         start=True, stop=True)
                    # evac (+ mask add broadcast over s) -> bf16
                    ev = evacp.tile([B, S], BF16, tag="a2evac")
                    if mask_allones:
                        if si == 0:
                            nc.vector.tensor_copy(out=ev[:], in_=ps[:])
                        else:
                            nc.scalar.copy(ev[:], ps[:])
                    else:
                        nc.vector.tensor_add(out=ev[:], in0=ps[:], in1=mask_sb[:])
                    # row-shuffle: [b, t] rows -> partition s of a2buf
                    nc.gpsimd.dma_start(a2buf[s:s + 1, :, :], ev[:])

            with (
                tc.tile_pool(name="mmps", bufs=2, space="PSUM") as mmps,
                tc.tile_pool(name="tps", bufs=2, space="PSUM") as tps,
                tc.tile_pool(name="a2ps", bufs=2, space="PSUM") as a2ps,
            ):
                # q first (attn2 needs all of qT), then k/v chunks woven
                # between attn_2 iterations so PE work covers kbT DMA waits
                for c in range(NQCH):
                    proj_chunk("q", c, mmps, tps)
                kv = [("k", c) for c in range(NPCH)] + [("v", c) for c in range(NPCH)]
                done = 0
                for g in range(SSL // 2):
                    attn2_group(g, a2ps)
                    want = ((g + 1) * len(kv)) // (SSL // 2)
                    while done < want:
                        proj_chunk(*kv[done], mmps, tps)
                        done += 1

            # ================= P3a: scores + softmax + eT, per b =================
            with (
                tc.tile_pool(name="scps", bufs=2, space="PSUM") as scps,
                tc.tile_pool(name="tps2", bufs=2, space="PSUM") as tps2,
            ):
                for b in range(B):
                    ps = scps.tile([128, S], F32, tag="sc")
                    for h in range(2):
                        sl = slice(h * 512, (h + 1) * 512)
                        nc.tensor.matmul(ps[:, sl], lhsT=qT_sb[:, b, :],
                                         rhs=kT_sb[:, b, sl], start=True, stop=False)
                        nc.tensor.matmul(ps[:, sl], lhsT=ident[:],
                                         rhs=a2buf[:, b, sl], start=False, stop=True)
                    e_sb = evacp.tile([128, S], BF16, tag="e", bufs=2)
                    nc.scalar.activation(e_sb[:], ps[:],
                                         mybir.ActivationFunctionType.Exp,
                                         bias=0.0, scale=1.0,
                                         accum_out=rowsum[:, b:b + 1])
                    for t in range(TC):
                        tp_ps = tps2.tile([128, 128], BF16, tag="tp2")
                        nc.tensor.transpose(tp_ps[:], e_sb[:, t * 128:(t + 1) * 128],
                                            ident[:])
                        nc.vector.tensor_copy(out=eT_sb[:, t, b, :], in_=tp_ps[:])
                nc.vector.reciprocal(recip[:], rowsum[:])

            # ================= P4: values_2 (bias values), per s =================
            # v2[b, d] = sum_t e[b, s, t] * v_bias[s, t, d]
            with tc.tile_pool(name="v2ps", bufs=4, space="PSUM") as v2ps:
                for g in range(SSL // 2):
                    vbt = streamp.tile([128, 2, TC, D], BF16, tag="vbt", bufs=2)
                    nc.sync.dma_start(vbt[:], vb[g])
                    for si in range(2):
                        s = 2 * g + si
                        ps = v2ps.tile([B, D], F32, tag="v2")
                        for t in range(TC):
                            nc.tensor.matmul(ps[:], lhsT=eT_sb[:, t, :, s],
                                             rhs=vbt[:, si, t, :],
                                             start=(t == 0), stop=(t == TC - 1))
                        ev = evacp.tile([B, D], BF16, tag="v2evac")
                        if si == 0:
                            nc.vector.tensor_copy(out=ev[:], in_=ps[:])
                        else:
                            nc.scalar.copy(ev[:], ps[:])
                        nc.gpsimd.dma_start(v2buf[s:s + 1, :, :], ev[:])

            # ================= P3b: values_1 + combine + out =================
            with tc.tile_pool(name="ops", bufs=2, space="PSUM") as ops:
                for b in range(B):
                    ps = ops.tile([128, D], F32, tag="o")
                    for t in range(TC):
                        nc.tensor.matmul(ps[:], lhsT=eT_sb[:, t, b, :],
                                         rhs=v_sb[:, b, t, :],
                                         start=(t == 0), stop=False)
                    nc.tensor.matmul(ps[:], lhsT=ident[:], rhs=v2buf[:, b, :],
                                     start=False, stop=True)
                    nc.scalar.activation(outbuf[:, b, :], ps[:],
                                         mybir.ActivationFunctionType.Copy,
                                         bias=0.0, scale=recip[:, b:b + 1])
                nc.sync.dma_start(out_h.rearrange("b s d -> s b d"), outbuf[:])

    nc.finalize()
    return nc


def _prep_inputs(query, key, value, mask, Wq, bq, Wk, bk, Wv, bv, k_bias, v_bias):
    scale = 1.0 / math.sqrt(D)
    bf = ml_dtypes.bfloat16
    keyT = np.ascontiguousarray(key.transpose(2, 0, 1).reshape(H, BS)).astype(bf)
    valueT = np.ascontiguousarray(value.transpose(2, 0, 1).reshape(H, BS)).astype(bf)
    WqTs = np.ascontiguousarray((Wq.T * scale)).astype(bf)
    WkT = np.ascontiguousarray(Wk.T).astype(bf)
    WvT = np.ascontiguousarray(Wv.T).astype(bf)
    bqs = (bq * scale).astype(np.float32)
    maskadd = np.where(mask == 0, np.float32(-1e9), np.float32(0.0)).astype(np.float32)

    in_maps = []
    for c in range(NCORES):
        ssl = slice(c * SSL, (c + 1) * SSL)
        queryT = np.ascontiguousarray(
            query[:, ssl, :].transpose(2, 0, 1).reshape(H, BSL)).astype(bf)
        kbT = np.ascontiguousarray(k_bias[ssl].transpose(0, 2, 1)).astype(bf)
        # vb: [s, t, d] -> [g, tp, sl, tc, d]  (s=2g+sl, t=tc*128+tp)
        vbc = np.ascontiguousarray(
            v_bias[ssl].reshape(SSL // 2, 2, TC, 128, D).transpose(0, 3, 1, 2, 4)
        ).astype(bf)
        in_maps.append(dict(
            queryT=queryT, keyT=keyT, valueT=valueT,
            WqT=WqTs, WkT=WkT, WvT=WvT,
            bq=bqs, bk=bk.astype(np.float32), bv=bv.astype(np.float32),
            kbT=kbT, vb=vbc, maskadd=maskadd,
        ))
    return in_maps


def kernel(**inputs):
    allones = bool((np.asarray(inputs["mask"]) != 0).all())
    key = f"nc{int(allones)}"
    if key not in _cache:
        _cache[key] = _build_nc(mask_allones=allones)
    nc = _cache[key]
    _cache["nc"] = nc
    in_maps = _prep_inputs(**{k: np.asarray(v) for k, v in inputs.items()})
    trace = bool(int(os.environ.get("ATTN_TRACE", "0")))
    res = run_bass_kernel_spmd(nc, in_maps, core_ids=list(range(NCORES)),
                               trace=trace)
    if trace and res.exec_time_ns is not None:
        print(f"HW exec time: {res.exec_time_ns} ns")
        _cache["exec_time_ns"] = res.exec_time_ns
        _cache["trace"] = res.instructions_and_trace
    out = np.concatenate([r["out"] for r in res.results], axis=1)
    return out
